# revision 52
# baseline (speedup 1.0000x reference)
"""AttentionBlock (GroupNorm + single-head self-attention + proj + residual)
on 8 trn2 NeuronCores.

Sharding: 8 cores = 4 batch elements x 2 query-halves. Each core computes
GroupNorm + full K/V for its batch element (duplicated across the 2 cores
sharing a batch, ~10% redundant FLOPs) and attention for its half of the
4096 tokens. Token order is rotated per-half on the host so every core runs
the identical NEFF on "its" tokens 0..2047 (SPMD, no collectives).

Device math (per core), all layouts channel-major [c_part, tok_free]:
  x [512, 4096] bf16 -> GN stats (per-partition sums + indicator-matmul for
  group sums) -> xn bf16 -> qkvT = W_eff @ xn (GN affine folded into
  weights host-side, bf16 matmul, fp32 PSUM) -> q/k/v cast to fp8e4 ->
  S^T pair tiles [2 x ktok, qtok] in 2-bank PSUM via fp8 DoubleRow matmuls
  (K=256/instr) -> one wide exp per pair on ScalarE with the 1/sqrt(c)
  score scale and a -2 shift folded in (keeps exp within e4m3 range;
  softmax shift-invariant) -> E^T fp8 -> softmax denominator via a chained
  DoubleRow ones-matmul whose [128,512] PSUM output holds den replicated
  across partitions (no DVE accumulation, no partition broadcast) ->
  attn@V fp8 DoubleRow with V token-major, retired to bf16 SBUF so PSUM
  banks recycle -> normalize to fp8 once 1/den lands -> proj fp8
  DoubleRow, software-pipelined one query-block late so the softmax tail
  hides under the next block's score matmuls -> + residual(+proj_bias,
  host-prefolded) -> out [512, 2048] f32.

Weight DMAs are issued from the ACT engine after the second GroupNorm
stats tile so they don't steal HBM bandwidth from the x load that gates
the critical path; the host pre-permutes them into one descriptor per
K/Q/V block.
"""

import sys

if "/opt/trn_rl_repo" not in sys.path:
    sys.path.insert(0, "/opt/trn_rl_repo")

import numpy as np
import ml_dtypes

import concourse.bass as bass
import concourse.bacc as bacc
import concourse.tile as tile
from concourse import mybir
from concourse.bass_utils import run_bass_kernel_spmd

F32 = mybir.dt.float32
BF16 = mybir.dt.bfloat16
F8 = mybir.dt.float8e4
AF = mybir.ActivationFunctionType
DR = mybir.MatmulPerfMode.DoubleRow

N, C, H, W = 4, 512, 64, 64
T = H * W            # 4096 tokens
TH = T // 2          # 2048 tokens per core
GROUPS = 32
GSIZE = C // GROUPS  # 16 channels per group
EPS = 1e-5
CT = C // 128        # 4 channel tiles
QB = TH // 512       # 4 query blocks of 512
KT = T // 128        # 32 key-token tiles
SCALE = 1.0 / np.sqrt(C)
C0 = 2.0             # exp shift: E = exp(S*SCALE - C0), keeps E in e4m3 range

_CACHE = {}


def _build(with_qkv_bias: bool):
    nc = bacc.Bacc("TRN2", target_bir_lowering=False, debug=False,
                   enable_asserts=False, num_devices=8)

    x_d = nc.dram_tensor("x", [C, T], F8, kind="ExternalInput")
    wk_d = nc.dram_tensor("wkT", [128, CT, C], F8, kind="ExternalInput")
    wq_d = nc.dram_tensor("wqT", [128, CT, C], F8, kind="ExternalInput")
    wv_d = nc.dram_tensor("wvT", [128, CT, C], F8, kind="ExternalInput")
    wp_d = nc.dram_tensor("wpT", [128, CT, C], F8, kind="ExternalInput")
    resid_d = nc.dram_tensor("resid", [C, TH], F32, kind="ExternalInput")
    ind_d = nc.dram_tensor("ind", [128, 128], F32, kind="ExternalInput")
    if with_qkv_bias:
        qb_d = nc.dram_tensor("qkv_bias", [128, 12], F32, kind="ExternalInput")
    out_d = nc.dram_tensor("out", [C, TH], F32, kind="ExternalOutput")

    with tile.TileContext(nc) as tc:
        with (
            tc.tile_pool(name="const", bufs=1) as cpool,
            tc.tile_pool(name="big", bufs=2) as bigpool,
            tc.tile_pool(name="kv", bufs=1) as kvpool,
            tc.tile_pool(name="xin", bufs=4) as xpool,
            tc.tile_pool(name="small", bufs=4) as spool,
            tc.tile_pool(name="attn", bufs=2) as apool,
            tc.tile_pool(name="io", bufs=3) as iopool,
            tc.tile_pool(name="psS", bufs=2, space="PSUM") as psS,
            tc.tile_pool(name="psB", bufs=2, space="PSUM") as psB,
            tc.tile_pool(name="psQ", bufs=2, space="PSUM") as psQ,
        ):
            # ---- constants (ind first: it gates the stats matmuls) ----
            ind_sb = cpool.tile([128, 128], F32)
            nc.sync.dma_start(out=ind_sb[:], in_=ind_d[:])
            ones8_sb = cpool.tile([128, 2, 128], F8)
            nc.vector.memset(ones8_sb[:], 1.0)
            onesb_sb = cpool.tile([128, 128], BF16)
            nc.vector.memset(onesb_sb[:], 1.0)
            ebias_sb = cpool.tile([128, 1], F32)
            nc.vector.memset(ebias_sb[:], -C0)
            # wkq_sb[:, ct, 0:C]=K cols, C:2C = Q cols; all qkv weights fp8
            wkq_sb = cpool.tile([128, CT, 2 * C], F8)
            wv_sb = cpool.tile([128, CT, C], F8)
            wp_sb = cpool.tile([128, CT, C], F8)
            if with_qkv_bias:
                qbias_sb = cpool.tile([128, 12], F32)
                nc.sync.dma_start(out=qbias_sb[:], in_=qb_d[:])

            # ---- GroupNorm -> xn (bf16, [128, CT, T]) ----
            # x/sq scratch borrows the kv pool's slots (kt/vt/qt are only
            # written in the qkv phase, after GN is done with the space).
            # One full-row DMA per channel tile; stats per full row;
            # normalize is spread across DVE/ACT/GpSimd.
            xn8 = kvpool.tile([128, CT, T], F8, tag="xn8")
            TH2 = T // 2
            # pass 1: per-channel-tile stats only, so ct3's stats run the
            # moment its x row lands instead of queueing behind normalize
            # work on the in-order DVE
            x_ts, stats = [], []
            for ct in range(CT):
                x_t = xpool.tile([128, T], F8, tag="x")
                nc.sync.dma_start(out=x_t[:],
                                  in_=x_d[ct * 128:(ct + 1) * 128, :])
                x_ts.append(x_t)
                s12 = spool.tile([128, 2], F32, tag="s12")
                sq_t = kvpool.tile([128, T], BF16, tag="qt")
                nc.scalar.activation(sq_t[:], x_t[:], AF.Square,
                                     accum_out=s12[:, 1:2])
                nc.vector.reduce_sum(s12[:, 0:1], x_t[:],
                                     axis=mybir.AxisListType.X)
                # group-sum across partitions via indicator matmul
                ps_pc = psQ.tile([128, 2], F32, tag="ps")
                nc.tensor.matmul(ps_pc[:], ind_sb[:], s12[:],
                                 start=True, stop=True)
                ms = spool.tile([128, 2], F32, tag="ms")
                nc.vector.tensor_scalar_mul(ms[:], ps_pc[:],
                                            1.0 / (GSIZE * T))
                stat = spool.tile([128, 4], F32, tag=f"stat{ct}")
                mean, var, rstd, nbias = (stat[:, i:i + 1] for i in range(4))
                nc.vector.tensor_mul(mean, ms[:, 0:1], ms[:, 0:1])
                nc.vector.tensor_sub(var, ms[:, 1:2], mean)
                nc.vector.tensor_scalar_add(var, var, EPS)
                nc.scalar.activation(var, var, AF.Sqrt)
                nc.vector.reciprocal(rstd, var)
                nc.vector.tensor_mul(nbias, ms[:, 0:1], rstd)
                nc.vector.tensor_scalar_mul(nbias, nbias, -1.0)
                stats.append((rstd, nbias))
                if ct == 1:
                    # weight DMAs gated behind the ct<=1 stats via canary
                    # writes into each destination tile (write-after-write
                    # hazards Tile can't hoist), so they don't steal HBM
                    # bandwidth from the x rows that gate the critical path
                    for canary in (wkq_sb[:, 0, 0:1], wkq_sb[:, 0, C:C + 1],
                                   wv_sb[:, 0, 0:1], wp_sb[:, 0, 0:1]):
                        nc.vector.tensor_copy(canary, stat[:, 0:1])
                    nc.sync.dma_start(out=wkq_sb[:, :, 0:C], in_=wk_d[:])
                    nc.sync.dma_start(out=wkq_sb[:, :, C:2 * C], in_=wq_d[:])
                    nc.sync.dma_start(out=wv_sb[:], in_=wv_d[:])
                    nc.sync.dma_start(out=wp_sb[:], in_=wp_d[:])
            # pass 2: normalize straight to fp8 (every qkv matmul is fp8
            # DoubleRow). ct0/ct1 go to GpSimd early (their stats land
            # first); ct3 — which gates the qkv chains — splits across
            # ACT and DVE the moment its stats land; ct2 fills both.
            def norm_chunk(eng, ct, h):
                rstd, nbias = stats[ct]
                sl = slice(h * TH2, (h + 1) * TH2)
                if eng == "act":
                    nc.scalar.activation(xn8[:, ct, sl], x_ts[ct][:, sl],
                                         AF.Identity, bias=nbias, scale=rstd)
                elif eng == "dve":
                    nc.vector.tensor_scalar(
                        xn8[:, ct, sl], x_ts[ct][:, sl], rstd, nbias,
                        mybir.AluOpType.mult, mybir.AluOpType.add)
                else:
                    nc.gpsimd.tensor_scalar(
                        xn8[:, ct, sl], x_ts[ct][:, sl], rstd, nbias,
                        mybir.AluOpType.mult, mybir.AluOpType.add)
            for eng, ct, h in (("gps", 0, 0), ("gps", 0, 1),
                               ("gps", 1, 0), ("gps", 1, 1),
                               ("act", 3, 0), ("dve", 3, 1),
                               ("dve", 2, 0), ("act", 2, 1)):
                norm_chunk(eng, ct, h)


            # ---- qkv projections (bf16 matmul -> fp8 tiles) ----
            # kT [c_head, tok] and qT [c_head, tok(half)], channel-major
            kt_sb = kvpool.tile([128, CT, T], F8, tag="kt")
            qt_sb = kvpool.tile([128, CT, TH], F8, tag="qt")
            vt_sb = kvpool.tile([128, KT, C], F8, tag="vt")
            for dk in range(CT):     # kT, fp8 DoubleRow
                for ts in range(T // 512):
                    ps = psQ.tile([128, 512], F32, tag="ps")
                    for cd in range(0, CT, 2):
                        nc.tensor.matmul(
                            ps[:],
                            wkq_sb[:, cd:cd + 2, dk * 128:(dk + 1) * 128],
                            xn8[:, cd:cd + 2, ts * 512:(ts + 1) * 512],
                            start=(cd == 0), stop=(cd == CT - 2),
                            perf_mode=DR)
                    if with_qkv_bias:
                        nc.scalar.activation(kt_sb[:, dk, ts * 512:(ts + 1) * 512],
                                             ps[:], AF.Identity,
                                             bias=qbias_sb[:, 4 + dk:5 + dk])
                    else:
                        nc.scalar.copy(kt_sb[:, dk, ts * 512:(ts + 1) * 512], ps[:])
            for dq in range(CT):     # qT: first TH tokens, fp8 DoubleRow
                for ts in range(TH // 512):
                    ps = psQ.tile([128, 512], F32, tag="ps")
                    for cd in range(0, CT, 2):
                        nc.tensor.matmul(
                            ps[:],
                            wkq_sb[:, cd:cd + 2, C + dq * 128:C + (dq + 1) * 128],
                            xn8[:, cd:cd + 2, ts * 512:(ts + 1) * 512],
                            start=(cd == 0), stop=(cd == CT - 2),
                            perf_mode=DR)
                    if with_qkv_bias:
                        nc.scalar.activation(qt_sb[:, dq, ts * 512:(ts + 1) * 512],
                                             ps[:], AF.Identity,
                                             bias=qbias_sb[:, dq:dq + 1])
                    else:
                        nc.scalar.copy(qt_sb[:, dq, ts * 512:(ts + 1) * 512], ps[:])
            for tv in range(KT):     # V token-major [tok, c], fp8 DoubleRow
                ps = psQ.tile([128, 512], F32, tag="ps")
                for cd in range(0, CT, 2):
                    nc.tensor.matmul(
                        ps[:],
                        xn8[:, cd:cd + 2, tv * 128:(tv + 1) * 128],
                        wv_sb[:, cd:cd + 2, :],
                        start=(cd == 0), stop=(cd == CT - 2),
                        perf_mode=DR)
                nc.vector.tensor_copy(vt_sb[:, tv, :], ps[:])

            # ---- attention, per query block of 512 (fp8 DoubleRow) ----
            def emit_proj(qb, at_sb):
                for co in range(CT):
                    ps_pr = psQ.tile([128, 512], F32, tag="ps")
                    for ci in range(0, CT, 2):
                        nc.tensor.matmul(
                            ps_pr[:],
                            wp_sb[:, ci:ci + 2, co * 128:(co + 1) * 128],
                            at_sb[:, ci:ci + 2, :],
                            start=(ci == 0), stop=(ci == CT - 2),
                            perf_mode=DR)
                    r_t = iopool.tile([128, 512], F32, tag="r")
                    nc.sync.dma_start(
                        out=r_t[:],
                        in_=resid_d[co * 128:(co + 1) * 128,
                                    qb * 512:(qb + 1) * 512])
                    o_t = iopool.tile([128, 512], F32, tag="o")
                    nc.vector.tensor_add(o_t[:], ps_pr[:], r_t[:])
                    nc.sync.dma_start(
                        out=out_d[co * 128:(co + 1) * 128,
                                  qb * 512:(qb + 1) * 512],
                        in_=o_t[:])

            at_prev = None
            for qb in range(QB):
                et = bigpool.tile([128, KT, 512], F8, tag="big")
                acc_g = apool.tile([128, 2, 512], BF16, tag="accg")
                for j in range(KT // 2):      # 16 key-tile pairs
                    kt2 = 2 * j
                    ps2 = psS.tile([128, 2, 512], F32, tag="s")
                    for half in range(2):
                        kt = kt2 + half
                        for cd in range(0, CT, 2):
                            nc.tensor.matmul(
                                ps2[:, half, :],
                                kt_sb[:, cd:cd + 2, kt * 128:(kt + 1) * 128],
                                qt_sb[:, cd:cd + 2, qb * 512:(qb + 1) * 512],
                                start=(cd == 0), stop=(cd == CT - 2),
                                perf_mode=DR)
                    # one wide exp over both PSUM banks
                    nc.scalar.activation(et[:, kt2:kt2 + 2, :], ps2[:, :, :],
                                         AF.Exp, bias=ebias_sb[:], scale=SCALE)
                    # odd pairs accumulate on the otherwise-idle GpSimd (NOT
                    # DVE — that delayed the av copies that recycle psB);
                    # even pairs stay on the TensorE den chain below
                    if j == 1:
                        nc.gpsimd.tensor_scalar(
                            acc_g[:], et[:, kt2:kt2 + 2, :], 1.0, 0.0,
                            mybir.AluOpType.mult, mybir.AluOpType.add)
                    elif j % 2 == 1:
                        nc.gpsimd.tensor_tensor(acc_g[:], acc_g[:],
                                                et[:, kt2:kt2 + 2, :],
                                                mybir.AluOpType.add)
                # proj for the previous query block: TensorE runs it here so
                # the previous block's softmax tail hides under our scores
                if at_prev is not None:
                    emit_proj(qb - 1, at_prev)
                # attnV: PSUM retires immediately to bf16 SBUF so psB banks
                # recycle without waiting on the softmax denominator. The
                # den ones-matmul chain (den replicated across partitions in
                # PSUM) is slotted after cv=1 so TensorE doesn't stall on
                # the exp tail; normalize muls run once 1/den lands and are
                # consumed by proj a query-block later.
                av_sb = apool.tile([128, CT, 512], BF16, tag="av")
                at_sb = apool.tile([128, CT, 512], F8, tag="at")
                rb = apool.tile([128, 512], F32, tag="rb")
                for cv in range(CT):
                    ps_av = psB.tile([128, 512], F32, tag="av")
                    for kt2 in range(0, KT, 2):
                        nc.tensor.matmul(
                            ps_av[:],
                            vt_sb[:, kt2:kt2 + 2, cv * 128:(cv + 1) * 128],
                            et[:, kt2:kt2 + 2, :],
                            start=(kt2 == 0), stop=(kt2 == KT - 2),
                            perf_mode=DR)
                    nc.vector.tensor_copy(av_sb[:, cv, :], ps_av[:])
                    if cv == 1:
                        ps_den = psQ.tile([128, 512], F32, tag="ps")
                        for jd in range(0, KT, 4):   # even pairs via DR
                            nc.tensor.matmul(
                                ps_den[:], ones8_sb[:],
                                et[:, jd:jd + 2, :],
                                start=(jd == 0), stop=False,
                                perf_mode=DR)
                        for jj in range(2):          # GpSimd-accumulated odds
                            nc.tensor.matmul(
                                ps_den[:], onesb_sb[:], acc_g[:, jj, :],
                                start=False, stop=(jj == 1))
                        nc.vector.reciprocal(rb[:], ps_den[:])
                for cv in range(CT):
                    nc.vector.tensor_mul(at_sb[:, cv, :], av_sb[:, cv, :],
                                         rb[:])
                at_prev = at_sb
            emit_proj(QB - 1, at_prev)

    nc.compile()
    return nc


def _prep_inputs(x, gn_weight, gn_bias, qkv_weight, proj_weight, proj_bias):
    """Host-side shard prep. Returns (in_maps, with_qkv_bias)."""
    bf16 = ml_dtypes.bfloat16
    f8 = ml_dtypes.float8_e4m3
    x, gn_weight, gn_bias, qkv_weight, proj_weight, proj_bias = (
        np.asarray(a) for a in
        (x, gn_weight, gn_bias, qkv_weight, proj_weight, proj_bias))
    xr = np.ascontiguousarray(x.reshape(N, C, T).astype(np.float32))
    # NOTE: the 1/sqrt(C) score scale is NOT folded into Wq here — it is
    # applied inside the exp activation on-device, which keeps q/k at unit
    # variance (the fp8 e4m3 sweet spot).
    w_eff = qkv_weight.astype(np.float64) * gn_weight.astype(np.float64)[None, :]
    qkv_bias = (qkv_weight.astype(np.float64) @ gn_bias.astype(np.float64))
    with_qkv_bias = bool(np.any(qkv_bias != 0.0))

    def wblock(wT, dt):
        # [C, C] (contraction-major) -> [128, CT, C] (partition, c-tile, col)
        return np.ascontiguousarray(
            wT.reshape(CT, 128, C).transpose(1, 0, 2).astype(dt))

    wqkvT = w_eff.T  # [C, 3C]
    wkT = wblock(wqkvT[:, C:2 * C], f8)
    wqT = wblock(wqkvT[:, 0:C], f8)
    wvT = wblock(wqkvT[:, 2 * C:3 * C], f8)
    wpT = wblock(proj_weight.T, f8)
    ind = (np.arange(128)[:, None] // GSIZE ==
           np.arange(128)[None, :] // GSIZE).astype(np.float32)
    in_maps = []
    for core in range(8):
        b, half = divmod(core, 2)
        xb = xr[b]
        if half:
            xb = np.ascontiguousarray(np.roll(xb, -TH, axis=1))
        resid = (xr[b][:, half * TH:(half + 1) * TH]
                 + proj_bias.astype(np.float32)[:, None])
        m = {"x": np.ascontiguousarray(xb.astype(f8)),
             "wkT": wkT, "wqT": wqT, "wvT": wvT, "wpT": wpT,
             "resid": np.ascontiguousarray(resid.astype(np.float32)),
             "ind": ind}
        if with_qkv_bias:
            m["qkv_bias"] = np.ascontiguousarray(
                qkv_bias.astype(np.float32).reshape(12, 128).T)
        in_maps.append(m)
    return in_maps, with_qkv_bias


def kernel(x, gn_weight, gn_bias, qkv_weight, proj_weight, proj_bias,
           _trace=False):
    in_maps, with_qkv_bias = _prep_inputs(
        x, gn_weight, gn_bias, qkv_weight, proj_weight, proj_bias)
    if with_qkv_bias not in _CACHE:
        _CACHE[with_qkv_bias] = _build(with_qkv_bias)
    nc = _CACHE[with_qkv_bias]
    res = run_bass_kernel_spmd(nc, in_maps, core_ids=list(range(8)),
                               trace=_trace)
    kernel.last_results = res
    out = np.empty((N, C, T), np.float32)
    for core in range(8):
        b, half = divmod(core, 2)
        out[b][:, half * TH:(half + 1) * TH] = res.results[core]["out"]
    return out.reshape(N, C, H, W)


# revision 53
# speedup vs baseline: 1.0021x; 1.0021x over previous
"""AttentionBlock (GroupNorm + single-head self-attention + proj + residual)
on 8 trn2 NeuronCores.

Sharding: 8 cores = 4 batch elements x 2 query-halves. Each core computes
GroupNorm + full K/V for its batch element (duplicated across the 2 cores
sharing a batch, ~10% redundant FLOPs) and attention for its half of the
4096 tokens. Token order is rotated per-half on the host so every core runs
the identical NEFF on "its" tokens 0..2047 (SPMD, no collectives).

Device math (per core), all layouts channel-major [c_part, tok_free]:
  x [512, 4096] bf16 -> GN stats (per-partition sums + indicator-matmul for
  group sums) -> xn bf16 -> qkvT = W_eff @ xn (GN affine folded into
  weights host-side, bf16 matmul, fp32 PSUM) -> q/k/v cast to fp8e4 ->
  S^T pair tiles [2 x ktok, qtok] in 2-bank PSUM via fp8 DoubleRow matmuls
  (K=256/instr) -> one wide exp per pair on ScalarE with the 1/sqrt(c)
  score scale and a -2 shift folded in (keeps exp within e4m3 range;
  softmax shift-invariant) -> E^T fp8 -> softmax denominator via a chained
  DoubleRow ones-matmul whose [128,512] PSUM output holds den replicated
  across partitions (no DVE accumulation, no partition broadcast) ->
  attn@V fp8 DoubleRow with V token-major, retired to bf16 SBUF so PSUM
  banks recycle -> normalize to fp8 once 1/den lands -> proj fp8
  DoubleRow, software-pipelined one query-block late so the softmax tail
  hides under the next block's score matmuls -> + residual(+proj_bias,
  host-prefolded) -> out [512, 2048] f32.

Weight DMAs are issued from the ACT engine after the second GroupNorm
stats tile so they don't steal HBM bandwidth from the x load that gates
the critical path; the host pre-permutes them into one descriptor per
K/Q/V block.
"""

import sys

if "/opt/trn_rl_repo" not in sys.path:
    sys.path.insert(0, "/opt/trn_rl_repo")

import numpy as np
import ml_dtypes

import concourse.bass as bass
import concourse.bacc as bacc
import concourse.tile as tile
from concourse import mybir
from concourse.bass_utils import run_bass_kernel_spmd

F32 = mybir.dt.float32
BF16 = mybir.dt.bfloat16
F8 = mybir.dt.float8e4
AF = mybir.ActivationFunctionType
DR = mybir.MatmulPerfMode.DoubleRow

N, C, H, W = 4, 512, 64, 64
T = H * W            # 4096 tokens
TH = T // 2          # 2048 tokens per core
GROUPS = 32
GSIZE = C // GROUPS  # 16 channels per group
EPS = 1e-5
CT = C // 128        # 4 channel tiles
QB = TH // 512       # 4 query blocks of 512
KT = T // 128        # 32 key-token tiles
SCALE = 1.0 / np.sqrt(C)
C0 = 2.0             # exp shift: E = exp(S*SCALE - C0), keeps E in e4m3 range

_CACHE = {}


def _build(with_qkv_bias: bool):
    nc = bacc.Bacc("TRN2", target_bir_lowering=False, debug=False,
                   enable_asserts=False, num_devices=8)

    x_d = nc.dram_tensor("x", [C, T], F8, kind="ExternalInput")
    wk_d = nc.dram_tensor("wkT", [128, CT, C], F8, kind="ExternalInput")
    wq_d = nc.dram_tensor("wqT", [128, CT, C], F8, kind="ExternalInput")
    wv_d = nc.dram_tensor("wvT", [128, CT, C], F8, kind="ExternalInput")
    wp_d = nc.dram_tensor("wpT", [128, CT, C], F8, kind="ExternalInput")
    resid_d = nc.dram_tensor("resid", [C, TH], F32, kind="ExternalInput")
    ind_d = nc.dram_tensor("ind", [128, 128], F32, kind="ExternalInput")
    if with_qkv_bias:
        qb_d = nc.dram_tensor("qkv_bias", [128, 12], F32, kind="ExternalInput")
    out_d = nc.dram_tensor("out", [C, TH], F32, kind="ExternalOutput")

    with tile.TileContext(nc) as tc:
        with (
            tc.tile_pool(name="const", bufs=1) as cpool,
            tc.tile_pool(name="big", bufs=2) as bigpool,
            tc.tile_pool(name="kv", bufs=1) as kvpool,
            tc.tile_pool(name="xin", bufs=4) as xpool,
            tc.tile_pool(name="small", bufs=4) as spool,
            tc.tile_pool(name="attn", bufs=2) as apool,
            tc.tile_pool(name="io", bufs=2) as iopool,
            tc.tile_pool(name="psS", bufs=2, space="PSUM") as psS,
            tc.tile_pool(name="psB", bufs=2, space="PSUM") as psB,
            tc.tile_pool(name="psQ", bufs=2, space="PSUM") as psQ,
        ):
            # ---- constants (ind first: it gates the stats matmuls) ----
            ind_sb = cpool.tile([128, 128], F32)
            nc.sync.dma_start(out=ind_sb[:], in_=ind_d[:])
            ones8_sb = cpool.tile([128, 2, 128], F8)
            nc.vector.memset(ones8_sb[:], 1.0)
            onesb_sb = cpool.tile([128, 128], BF16)
            nc.vector.memset(onesb_sb[:], 1.0)
            ebias_sb = cpool.tile([128, 1], F32)
            nc.vector.memset(ebias_sb[:], -C0)
            # wkq_sb[:, ct, 0:C]=K cols, C:2C = Q cols; all qkv weights fp8
            wkq_sb = cpool.tile([128, CT, 2 * C], F8)
            wv_sb = cpool.tile([128, CT, C], F8)
            wp_sb = cpool.tile([128, CT, C], F8)
            if with_qkv_bias:
                qbias_sb = cpool.tile([128, 12], F32)
                nc.sync.dma_start(out=qbias_sb[:], in_=qb_d[:])

            # ---- GroupNorm -> xn (bf16, [128, CT, T]) ----
            # x/sq scratch borrows the kv pool's slots (kt/vt/qt are only
            # written in the qkv phase, after GN is done with the space).
            # One full-row DMA per channel tile; stats per full row;
            # normalize is spread across DVE/ACT/GpSimd.
            xn8 = kvpool.tile([128, CT, T], F8, tag="xn8")
            TH2 = T // 2
            # pass 1: per-channel-tile stats only, so ct3's stats run the
            # moment its x row lands instead of queueing behind normalize
            # work on the in-order DVE
            x_ts, stats = [], []
            for ct in range(CT):
                x_t = xpool.tile([128, T], F8, tag="x")
                nc.sync.dma_start(out=x_t[:],
                                  in_=x_d[ct * 128:(ct + 1) * 128, :])
                x_ts.append(x_t)
                s12 = spool.tile([128, 2], F32, tag="s12")
                sq_t = kvpool.tile([128, T], BF16, tag="qt")
                nc.scalar.activation(sq_t[:], x_t[:], AF.Square,
                                     accum_out=s12[:, 1:2])
                nc.vector.reduce_sum(s12[:, 0:1], x_t[:],
                                     axis=mybir.AxisListType.X)
                # group-sum across partitions via indicator matmul
                ps_pc = psQ.tile([128, 2], F32, tag="ps")
                nc.tensor.matmul(ps_pc[:], ind_sb[:], s12[:],
                                 start=True, stop=True)
                ms = spool.tile([128, 2], F32, tag="ms")
                nc.vector.tensor_scalar_mul(ms[:], ps_pc[:],
                                            1.0 / (GSIZE * T))
                stat = spool.tile([128, 4], F32, tag=f"stat{ct}")
                mean, var, rstd, nbias = (stat[:, i:i + 1] for i in range(4))
                nc.vector.tensor_mul(mean, ms[:, 0:1], ms[:, 0:1])
                nc.vector.tensor_sub(var, ms[:, 1:2], mean)
                nc.vector.tensor_scalar_add(var, var, EPS)
                nc.scalar.activation(var, var, AF.Sqrt)
                nc.vector.reciprocal(rstd, var)
                nc.vector.tensor_mul(nbias, ms[:, 0:1], rstd)
                nc.vector.tensor_scalar_mul(nbias, nbias, -1.0)
                stats.append((rstd, nbias))
                if ct == 1:
                    # weight DMAs gated behind the ct<=1 stats via canary
                    # writes into each destination tile (write-after-write
                    # hazards Tile can't hoist), so they don't steal HBM
                    # bandwidth from the x rows that gate the critical path
                    for canary in (wkq_sb[:, 0, 0:1], wkq_sb[:, 0, C:C + 1],
                                   wv_sb[:, 0, 0:1], wp_sb[:, 0, 0:1]):
                        nc.vector.tensor_copy(canary, stat[:, 0:1])
                    nc.sync.dma_start(out=wkq_sb[:, :, 0:C], in_=wk_d[:])
                    nc.sync.dma_start(out=wkq_sb[:, :, C:2 * C], in_=wq_d[:])
                    nc.sync.dma_start(out=wv_sb[:], in_=wv_d[:])
                    nc.sync.dma_start(out=wp_sb[:], in_=wp_d[:])
            # pass 2: normalize straight to fp8 (every qkv matmul is fp8
            # DoubleRow). ct0/ct1 go to GpSimd early (their stats land
            # first); ct3 — which gates the qkv chains — splits across
            # ACT and DVE the moment its stats land; ct2 fills both.
            def norm_chunk(eng, ct, h):
                rstd, nbias = stats[ct]
                sl = slice(h * TH2, (h + 1) * TH2)
                if eng == "act":
                    nc.scalar.activation(xn8[:, ct, sl], x_ts[ct][:, sl],
                                         AF.Identity, bias=nbias, scale=rstd)
                elif eng == "dve":
                    nc.vector.tensor_scalar(
                        xn8[:, ct, sl], x_ts[ct][:, sl], rstd, nbias,
                        mybir.AluOpType.mult, mybir.AluOpType.add)
                else:
                    nc.gpsimd.tensor_scalar(
                        xn8[:, ct, sl], x_ts[ct][:, sl], rstd, nbias,
                        mybir.AluOpType.mult, mybir.AluOpType.add)
            for eng, ct, h in (("gps", 0, 0), ("gps", 0, 1),
                               ("gps", 1, 0), ("gps", 1, 1),
                               ("act", 3, 0), ("dve", 3, 1),
                               ("dve", 2, 0), ("act", 2, 1)):
                norm_chunk(eng, ct, h)


            # ---- qkv projections (bf16 matmul -> fp8 tiles) ----
            # kT [c_head, tok] and qT [c_head, tok(half)], channel-major
            kt_sb = kvpool.tile([128, CT, T], F8, tag="kt")
            qt_sb = kvpool.tile([128, CT, TH], F8, tag="qt")
            vt_sb = kvpool.tile([128, KT, C], F8, tag="vt")
            for dk in range(CT):     # kT, fp8 DoubleRow
                for ts in range(T // 512):
                    ps = psQ.tile([128, 512], F32, tag="ps")
                    for cd in range(0, CT, 2):
                        nc.tensor.matmul(
                            ps[:],
                            wkq_sb[:, cd:cd + 2, dk * 128:(dk + 1) * 128],
                            xn8[:, cd:cd + 2, ts * 512:(ts + 1) * 512],
                            start=(cd == 0), stop=(cd == CT - 2),
                            perf_mode=DR)
                    if with_qkv_bias:
                        nc.scalar.activation(kt_sb[:, dk, ts * 512:(ts + 1) * 512],
                                             ps[:], AF.Identity,
                                             bias=qbias_sb[:, 4 + dk:5 + dk])
                    else:
                        nc.scalar.copy(kt_sb[:, dk, ts * 512:(ts + 1) * 512], ps[:])
            for dq in range(CT):     # qT: first TH tokens, fp8 DoubleRow
                for ts in range(TH // 512):
                    ps = psQ.tile([128, 512], F32, tag="ps")
                    for cd in range(0, CT, 2):
                        nc.tensor.matmul(
                            ps[:],
                            wkq_sb[:, cd:cd + 2, C + dq * 128:C + (dq + 1) * 128],
                            xn8[:, cd:cd + 2, ts * 512:(ts + 1) * 512],
                            start=(cd == 0), stop=(cd == CT - 2),
                            perf_mode=DR)
                    if with_qkv_bias:
                        nc.scalar.activation(qt_sb[:, dq, ts * 512:(ts + 1) * 512],
                                             ps[:], AF.Identity,
                                             bias=qbias_sb[:, dq:dq + 1])
                    else:
                        nc.scalar.copy(qt_sb[:, dq, ts * 512:(ts + 1) * 512], ps[:])
            for tv in range(KT):     # V token-major [tok, c], fp8 DoubleRow
                ps = psQ.tile([128, 512], F32, tag="ps")
                for cd in range(0, CT, 2):
                    nc.tensor.matmul(
                        ps[:],
                        xn8[:, cd:cd + 2, tv * 128:(tv + 1) * 128],
                        wv_sb[:, cd:cd + 2, :],
                        start=(cd == 0), stop=(cd == CT - 2),
                        perf_mode=DR)
                nc.vector.tensor_copy(vt_sb[:, tv, :], ps[:])

            # ---- attention, per query block of 512 (fp8 DoubleRow) ----
            def emit_proj(qb, at_sb):
                for co in range(CT):
                    ps_pr = psQ.tile([128, 512], F32, tag="ps")
                    for ci in range(0, CT, 2):
                        nc.tensor.matmul(
                            ps_pr[:],
                            wp_sb[:, ci:ci + 2, co * 128:(co + 1) * 128],
                            at_sb[:, ci:ci + 2, :],
                            start=(ci == 0), stop=(ci == CT - 2),
                            perf_mode=DR)
                    r_t = iopool.tile([128, 512], F32, tag="r")
                    nc.sync.dma_start(
                        out=r_t[:],
                        in_=resid_d[co * 128:(co + 1) * 128,
                                    qb * 512:(qb + 1) * 512])
                    o_t = iopool.tile([128, 512], F32, tag="o")
                    nc.vector.tensor_add(o_t[:], ps_pr[:], r_t[:])
                    nc.sync.dma_start(
                        out=out_d[co * 128:(co + 1) * 128,
                                  qb * 512:(qb + 1) * 512],
                        in_=o_t[:])

            at_prev = None
            for qb in range(QB):
                et = bigpool.tile([128, KT, 512], F8, tag="big")
                acc_g = apool.tile([128, 2, 512], BF16, tag="accg")
                for j in range(KT // 2):      # 16 key-tile pairs
                    kt2 = 2 * j
                    ps2 = psS.tile([128, 2, 512], F32, tag="s")
                    for half in range(2):
                        kt = kt2 + half
                        for cd in range(0, CT, 2):
                            nc.tensor.matmul(
                                ps2[:, half, :],
                                kt_sb[:, cd:cd + 2, kt * 128:(kt + 1) * 128],
                                qt_sb[:, cd:cd + 2, qb * 512:(qb + 1) * 512],
                                start=(cd == 0), stop=(cd == CT - 2),
                                perf_mode=DR)
                    # one wide exp over both PSUM banks
                    nc.scalar.activation(et[:, kt2:kt2 + 2, :], ps2[:, :, :],
                                         AF.Exp, bias=ebias_sb[:], scale=SCALE)
                    # odd pairs accumulate on the otherwise-idle GpSimd (NOT
                    # DVE — that delayed the av copies that recycle psB);
                    # even pairs stay on the TensorE den chain below
                    if j == 1:
                        nc.gpsimd.tensor_scalar(
                            acc_g[:], et[:, kt2:kt2 + 2, :], 1.0, 0.0,
                            mybir.AluOpType.mult, mybir.AluOpType.add)
                    elif j % 2 == 1:
                        nc.gpsimd.tensor_tensor(acc_g[:], acc_g[:],
                                                et[:, kt2:kt2 + 2, :],
                                                mybir.AluOpType.add)
                # proj for the previous query block: TensorE runs it here so
                # the previous block's softmax tail hides under our scores
                if at_prev is not None:
                    emit_proj(qb - 1, at_prev)
                # attnV: PSUM retires immediately to bf16 SBUF so psB banks
                # recycle without waiting on the softmax denominator. The
                # den ones-matmul chain (den replicated across partitions in
                # PSUM) is slotted after cv=1 so TensorE doesn't stall on
                # the exp tail; normalize muls run once 1/den lands and are
                # consumed by proj a query-block later.
                av_sb = apool.tile([128, CT, 512], BF16, tag="av")
                at_sb = apool.tile([128, CT, 512], F8, tag="at")
                rb = apool.tile([128, 512], F32, tag="rb")
                for cv in range(CT):
                    ps_av = psB.tile([128, 512], F32, tag="av")
                    for kt2 in range(0, KT, 2):
                        nc.tensor.matmul(
                            ps_av[:],
                            vt_sb[:, kt2:kt2 + 2, cv * 128:(cv + 1) * 128],
                            et[:, kt2:kt2 + 2, :],
                            start=(kt2 == 0), stop=(kt2 == KT - 2),
                            perf_mode=DR)
                    nc.vector.tensor_copy(av_sb[:, cv, :], ps_av[:])
                    if cv == 1:
                        ps_den = psQ.tile([128, 512], F32, tag="ps")
                        for jd in range(0, KT, 4):   # even pairs via DR
                            nc.tensor.matmul(
                                ps_den[:], ones8_sb[:],
                                et[:, jd:jd + 2, :],
                                start=(jd == 0), stop=False,
                                perf_mode=DR)
                        for jj in range(2):          # GpSimd-accumulated odds
                            nc.tensor.matmul(
                                ps_den[:], onesb_sb[:], acc_g[:, jj, :],
                                start=False, stop=(jj == 1))
                        nc.vector.reciprocal(rb[:], ps_den[:])
                for cv in range(CT):
                    nc.vector.tensor_mul(at_sb[:, cv, :], av_sb[:, cv, :],
                                         rb[:])
                at_prev = at_sb
            emit_proj(QB - 1, at_prev)

    nc.compile()
    return nc


def _prep_inputs(x, gn_weight, gn_bias, qkv_weight, proj_weight, proj_bias):
    """Host-side shard prep. Returns (in_maps, with_qkv_bias)."""
    bf16 = ml_dtypes.bfloat16
    f8 = ml_dtypes.float8_e4m3
    x, gn_weight, gn_bias, qkv_weight, proj_weight, proj_bias = (
        np.asarray(a) for a in
        (x, gn_weight, gn_bias, qkv_weight, proj_weight, proj_bias))
    xr = np.ascontiguousarray(x.reshape(N, C, T).astype(np.float32))
    # NOTE: the 1/sqrt(C) score scale is NOT folded into Wq here — it is
    # applied inside the exp activation on-device, which keeps q/k at unit
    # variance (the fp8 e4m3 sweet spot).
    w_eff = qkv_weight.astype(np.float64) * gn_weight.astype(np.float64)[None, :]
    qkv_bias = (qkv_weight.astype(np.float64) @ gn_bias.astype(np.float64))
    with_qkv_bias = bool(np.any(qkv_bias != 0.0))

    def wblock(wT, dt):
        # [C, C] (contraction-major) -> [128, CT, C] (partition, c-tile, col)
        return np.ascontiguousarray(
            wT.reshape(CT, 128, C).transpose(1, 0, 2).astype(dt))

    wqkvT = w_eff.T  # [C, 3C]
    wkT = wblock(wqkvT[:, C:2 * C], f8)
    wqT = wblock(wqkvT[:, 0:C], f8)
    wvT = wblock(wqkvT[:, 2 * C:3 * C], f8)
    wpT = wblock(proj_weight.T, f8)
    ind = (np.arange(128)[:, None] // GSIZE ==
           np.arange(128)[None, :] // GSIZE).astype(np.float32)
    in_maps = []
    for core in range(8):
        b, half = divmod(core, 2)
        xb = xr[b]
        if half:
            xb = np.ascontiguousarray(np.roll(xb, -TH, axis=1))
        resid = (xr[b][:, half * TH:(half + 1) * TH]
                 + proj_bias.astype(np.float32)[:, None])
        m = {"x": np.ascontiguousarray(xb.astype(f8)),
             "wkT": wkT, "wqT": wqT, "wvT": wvT, "wpT": wpT,
             "resid": np.ascontiguousarray(resid.astype(np.float32)),
             "ind": ind}
        if with_qkv_bias:
            m["qkv_bias"] = np.ascontiguousarray(
                qkv_bias.astype(np.float32).reshape(12, 128).T)
        in_maps.append(m)
    return in_maps, with_qkv_bias


def kernel(x, gn_weight, gn_bias, qkv_weight, proj_weight, proj_bias,
           _trace=False):
    in_maps, with_qkv_bias = _prep_inputs(
        x, gn_weight, gn_bias, qkv_weight, proj_weight, proj_bias)
    if with_qkv_bias not in _CACHE:
        _CACHE[with_qkv_bias] = _build(with_qkv_bias)
    nc = _CACHE[with_qkv_bias]
    res = run_bass_kernel_spmd(nc, in_maps, core_ids=list(range(8)),
                               trace=_trace)
    kernel.last_results = res
    out = np.empty((N, C, T), np.float32)
    for core in range(8):
        b, half = divmod(core, 2)
        out[b][:, half * TH:(half + 1) * TH] = res.results[core]["out"]
    return out.reshape(N, C, H, W)
